# revision 34
# baseline (speedup 1.0000x reference)
"""Trainium2 Bass kernel for nn_EncodingNetwork (gnn_message_passing).

Math (exact collapse of the reference):
    enc       = x @ W_enc.T + b_enc                    [N=200, D=1024]
    cm[w]     = class-mean of enc = xm[w] @ W_enc.T + b_enc   (xm = class-mean of x)
    gm        = mean(enc, axis=0) = mean(cm, axis=0)
    per_class = cm @ Wl.T + gm @ Wr.T + b_rel          [20, 2D]
    out       = gaussian * per_class[:, D:] + per_class[:, :D]

Sharding across 8 cores: the final 1024 output columns are split 128/core.
Each core loads only its 128-row slices of W_rel (2 MB instead of 16 MB) and
a 1024x128 column slice of W_enc (0.5 MB).  The class-mean matrix cm is
computed as a per-core 128-column slice and assembled with an 8-core
AllGather.  Everything on device runs in transposed [feature, class] layout
so the contraction dim always sits on SBUF partitions.

Implementation notes (hard-won on this toolchain):
  - bacc.Bacc + nc.finalize() are required: raw bass.Bass programs emit
    multi-semaphore waits that walrus codegen rejects ("Too many sync wait
    commands"); Bacc legalizes them.
  - Big loads must go through the HW-DGE rings (nc.sync / nc.scalar):
    SWDGE (gpsimd) generates descriptors in ucode at ~1us each, which
    serializes a [128, N] load into ~100 us of trickle.
  - Inputs are host-packed into three blobs so the pre-AllGather path
    (x + selector + smalls + W_enc slice) lands early while the 2 MB
    W_rel slice streams in parallel.
"""

import numpy as np

import concourse.bass as bass  # noqa: F401
import concourse.tile as tile
from concourse import bacc, mybir
from concourse.bass import ts
from concourse.bass_utils import run_bass_kernel_spmd

N_WAY = 20
N_SUPPORT = 10
N = N_WAY * N_SUPPORT  # 200
D = 1024
NC = 8
SL = D // NC  # 128 output columns per core
KT = D // 128  # 8 contraction tiles
XW = D + N_WAY  # x | selector columns, per 128-row tile
F32 = mybir.dt.float32

USE_ALLGATHER = False
# PE matmul operand dtype: "f32" (exact, two half-speed passes), "f32r"
# (single-pass fp32 with relaxed multiply precision, ~2.6e-4 absmax-rel),
# or "bf16" (halved DMA + fast weight load, ~1e-3 absmax-rel).  The
# epilogue (biases, gaussian combine) always runs in fp32.
MM_DTYPE = "bf16"


def _build_nc(use_ag: bool) -> bacc.Bacc:
    nc = bacc.Bacc("TRN2", target_bir_lowering=False, debug=False, num_devices=NC)

    FD = {
        "f32": F32,
        "f32r": mybir.dt.float32r,
        "bf16": mybir.dt.bfloat16,
    }[MM_DTYPE]

    def mm(out, lhsT, rhs, **kw):
        nc.tensor.matmul(out, lhsT, rhs, **kw)

    # wcm per contraction tile kt: the W_enc^T chunk ([128, SL] for the AG
    # variant, [128, D] replicated otherwise).
    # wrel per contraction tile kt, four 128-wide blocks:
    #   A = Wl^T chunk for the means rows   (W_rel[S_c, :D])
    #   B = Wr^T chunk for the means rows   (W_rel[S_c, D:])
    #   C = Wl^T chunk for the stds rows    (W_rel[D+S_c, :D])
    #   Dd= Wr^T chunk for the stds rows    (W_rel[D+S_c, D:])
    wenc_w = SL if use_ag else D

    xs_h = nc.declare_dram_parameter("xsb", [128, 2 * XW], FD, isOutput=False)
    wcm_h = nc.declare_dram_parameter("wcm", [128, KT * wenc_w], FD, isOutput=False)
    wrel_h = nc.declare_dram_parameter("wrel", [128, KT * 512], FD, isOutput=False)
    sml_h = nc.declare_dram_parameter("sml", [128, 31], F32, isOutput=False)
    out_h = nc.declare_dram_parameter("out", [128, N_WAY], F32, isOutput=True)

    if use_ag:
        cc_in = nc.dram_tensor("cc_in", [128, N_WAY], F32)
        cc_out = nc.dram_tensor("cc_out", [D, N_WAY], F32, addr_space="Shared")

    with tile.TileContext(nc) as tc:
        with (
            tc.tile_pool(name="sbuf", bufs=1) as sb,
            tc.tile_pool(name="psum1", bufs=1, space="PSUM") as ps,
            tc.tile_pool(name="psum2", bufs=2, space="PSUM") as ps2,
        ):
            # ---- loads: HW-DGE rings. The PE chain is xm -> cm -> rel, so
            # W_enc (cm's weights, the big blob) streams first on the SP
            # ring (which starts ~3us earlier than Act); the x blob rides
            # the Act ring and lands in time for xm; W_rel follows.  Keep
            # per-DMA descriptors large (>=16KB/partition runs full rate).
            xs_all = sb.tile([128, 2 * XW], FD, tag="xs")
            nc.sync.dma_start(xs_all[:], xs_h[:])
            wcm_all = sb.tile([128, KT * wenc_w], FD, tag="wcm")
            wrel_all = sb.tile([128, KT * 512], FD, tag="wrel")
            if use_ag:
                nc.sync.dma_start(wcm_all[:], wcm_h[:])
                nc.scalar.dma_start(wrel_all[:], wrel_h[:])
            else:
                # wcm is t-major ([t][kt][128] blocks): half A holds the
                # weights for m-chunks 0-3, half B for 4-7.  B rides the
                # Act ring (which has no x blob ahead of it) so it lands
                # first; the cm passes run 4..7 then 0..3 to match.
                half = KT * wenc_w // 2
                nc.sync.dma_start(wcm_all[:, :half], wcm_h[:, :half])
                nc.scalar.dma_start(wcm_all[:, half:], wcm_h[:, half:])
                nc.scalar.dma_start(wrel_all[:], wrel_h[:])
            smw = sb.tile([128, 31], F32, tag="smw")
            nc.sync.dma_start(smw[:], sml_h[:])

            # ---- stages 1+2 interleaved.
            # stage 1: xm^T [d, w] = x^T @ S, one 128-row chunk per k-tile.
            # stage 2 (no-AG): cm^T = W_enc^T-chunks @ xm^T (+ b_enc), with
            # the k-tile loop OUTERMOST in the first half so each cm
            # contraction step runs right after its xm chunk and its W_enc
            # chunk land -- the PE chain starts ~8us earlier than a serial
            # xm-then-cm ordering.  PSUM budget: 4 cm accumulators + 2 xm
            # tiles + 1-2 rel-era tiles <= 8 banks, so cm runs in two
            # 4-m-chunk passes.
            xm_sb = sb.tile([128, KT * N_WAY], FD, tag="xm")
            cmf_sb = sb.tile([128, KT * N_WAY], FD, tag="cmf")

            def emit_xm(t):
                p = ps2.tile([128, N_WAY], F32, tag="xm_ps")
                for i in range(2):
                    mm(
                        p[:],
                        xs_all[:, i * XW + t * 128 : i * XW + (t + 1) * 128],
                        xs_all[:, i * XW + D : i * XW + D + N_WAY],
                        start=(i == 0),
                        stop=(i == 1),
                    )
                nc.vector.tensor_copy(xm_sb[:, ts(t, N_WAY)], p[:])

            if use_ag:
                for t in range(KT):
                    emit_xm(t)

                pcm = ps.tile([128, N_WAY], F32, tag="cm_ps")
                for kt in range(KT):
                    mm(
                        pcm[:],
                        wcm_all[:, ts(kt, SL)],
                        xm_sb[:, ts(kt, N_WAY)],
                        start=(kt == 0),
                        stop=(kt == KT - 1),
                    )
                cm_own = sb.tile([128, N_WAY], F32, tag="cm_own")
                nc.vector.tensor_copy(cm_own[:], pcm[:])
                nc.vector.tensor_add(
                    cm_own[:], cm_own[:], smw[:, 30:31].broadcast_to((128, N_WAY))
                )
                # ---- stage 3: AllGather the 8 column slices of cm^T
                nc.sync.dma_start(cc_in[:], cm_own[:])
                nc.gpsimd.collective_compute(
                    "AllGather",
                    mybir.AluOpType.bypass,
                    replica_groups=[list(range(NC))],
                    ins=[cc_in[:]],
                    outs=[cc_out[:]],
                )
                nc.sync.dma_start(
                    cmf_sb[:].rearrange("p (t w) -> p t w", t=KT),
                    cc_out[:].rearrange("(t p) w -> p t w", p=128),
                )
            else:
                for t in range(KT):
                    emit_xm(t)
                order = list(range(4, KT)) + list(range(4))
                pm = ps.tile([128, N_WAY], F32, tag="pm")
                pmR = ps.tile([128, N_WAY], F32, tag="pmR")
                pstd = ps.tile([128, N_WAY], F32, tag="pstd")
                pstdR = ps.tile([128, N_WAY], F32, tag="pstdR")
                for i, t in enumerate(order):
                    pcm = ps.tile(
                        [128, N_WAY], F32, tag=f"cm_ps{t % 2}", name=f"pcm{t}"
                    )
                    for kt in range(KT):
                        mm(
                            pcm[:],
                            wcm_all[:, t * D + kt * 128 : t * D + (kt + 1) * 128],
                            xm_sb[:, ts(kt, N_WAY)],
                            start=(kt == 0),
                            stop=(kt == KT - 1),
                        )
                    nc.vector.tensor_copy(cmf_sb[:, ts(t, N_WAY)], pcm[:])
                    nc.vector.tensor_add(
                        cmf_sb[:, ts(t, N_WAY)],
                        cmf_sb[:, ts(t, N_WAY)],
                        smw[:, t : t + 1].broadcast_to((128, N_WAY)),
                    )
                    # rel accumulation step for contraction chunk t runs as
                    # soon as its cmf chunk exists -- no serial rel block.
                    rhs = cmf_sb[:, ts(t, N_WAY)]
                    st, sp = (i == 0), (i == KT - 1)
                    o = t * 512
                    mm(pm[:], wrel_all[:, o : o + 128], rhs, start=st, stop=sp)
                    mm(pmR[:], wrel_all[:, o + 128 : o + 256], rhs, start=st, stop=sp)
                    mm(pstd[:], wrel_all[:, o + 256 : o + 384], rhs, start=st, stop=sp)
                    mm(pstdR[:], wrel_all[:, o + 384 : o + 512], rhs, start=st, stop=sp)

            # ---- stage 4 (AG variant only; no-AG interleaves rel above)
            if use_ag:
                pm = ps.tile([128, N_WAY], F32, tag="pm")
                pmR = ps.tile([128, N_WAY], F32, tag="pmR")
                pstd = ps.tile([128, N_WAY], F32, tag="pstd")
                pstdR = ps.tile([128, N_WAY], F32, tag="pstdR")
                for kt in range(KT):
                    rhs = cmf_sb[:, ts(kt, N_WAY)]
                    st, sp = (kt == 0), (kt == KT - 1)
                    o = kt * 512
                    mm(pm[:], wrel_all[:, o : o + 128], rhs, start=st, stop=sp)
                    mm(pmR[:], wrel_all[:, o + 128 : o + 256], rhs, start=st, stop=sp)
                    mm(pstd[:], wrel_all[:, o + 256 : o + 384], rhs, start=st, stop=sp)
                    mm(pstdR[:], wrel_all[:, o + 384 : o + 512], rhs, start=st, stop=sp)

            # ---- stage 5: fold the rhs-term row-means + biases, combine
            rm = sb.tile([128, 1], F32, tag="rm")
            rs = sb.tile([128, 1], F32, tag="rs")
            nc.vector.reduce_sum(rm[:], pmR[:], axis=mybir.AxisListType.X)
            nc.vector.reduce_sum(rs[:], pstdR[:], axis=mybir.AxisListType.X)
            bias_m = sb.tile([128, 1], F32, tag="bias_m")
            bias_s = sb.tile([128, 1], F32, tag="bias_s")
            nc.vector.tensor_scalar(
                bias_m[:], rm[:], 1.0 / N_WAY, smw[:, 8:9],
                op0=mybir.AluOpType.mult, op1=mybir.AluOpType.add,
            )
            nc.vector.tensor_scalar(
                bias_s[:], rs[:], 1.0 / N_WAY, smw[:, 9:10],
                op0=mybir.AluOpType.mult, op1=mybir.AluOpType.add,
            )
            t_sg = sb.tile([128, N_WAY], F32, tag="t_sg")
            nc.vector.scalar_tensor_tensor(
                t_sg[:], pstd[:], bias_s[:], smw[:, 10:30],
                op0=mybir.AluOpType.add, op1=mybir.AluOpType.mult,
            )
            out_sb = sb.tile([128, N_WAY], F32, tag="out")
            nc.vector.scalar_tensor_tensor(
                out_sb[:], pm[:], bias_m[:], t_sg[:],
                op0=mybir.AluOpType.add, op1=mybir.AluOpType.add,
            )
            nc.sync.dma_start(out_h[:], out_sb[:])

    nc.finalize()
    return nc


_NC_CACHE: dict = {}


def _get_nc(use_ag: bool) -> bacc.Bacc:
    key = (use_ag, MM_DTYPE)
    if key not in _NC_CACHE:
        _NC_CACHE[key] = _build_nc(use_ag)
    return _NC_CACHE[key]


def _np_dtype():
    if MM_DTYPE == "bf16":
        import ml_dtypes

        return ml_dtypes.bfloat16
    return np.float32


def _make_in_maps(x, W_enc, b_enc, W_rel, b_rel, gaussian, use_ag):
    nd = _np_dtype()
    # The class-mean scaling 1/N_SUPPORT is folded into W_enc on the host
    # (in fp32, before any cast) so the selector stays exactly 1.0.
    W_enc = W_enc / np.float32(N_SUPPORT)
    # xsb: [128, 2*XW] — two 128-row tiles of [x | selector]
    xs = np.zeros((2, 128, XW), np.float32)
    xs[:, :, :D].reshape(256, D)[:N] = x
    sel = np.zeros((N, N_WAY), np.float32)
    sel[np.arange(N), np.arange(N) // N_SUPPORT] = 1.0
    xs[:, :, D : D + N_WAY].reshape(256, N_WAY)[:N] = sel

    in_maps = []
    for c in range(NC):
        s = slice(c * SL, (c + 1) * SL)
        s2 = slice(D + c * SL, D + (c + 1) * SL)
        if use_ag:
            # W_enc^T column slice: [D, SL] -> [128, KT*SL] chunk-interleaved
            wcm = (
                np.ascontiguousarray(W_enc[s, :].T)
                .reshape(KT, 128, SL)
                .transpose(1, 0, 2)
                .reshape(128, KT * SL)
            )
        else:
            # t-major blocks: wcm[p, t*D + kt*128 + j] = W_enc[t*128+j, kt*128+p]
            wcm = (
                np.ascontiguousarray(W_enc.T)
                .reshape(KT, 128, KT, 128)
                .transpose(1, 2, 0, 3)
                .reshape(128, KT * D)
            )
        blk = np.empty((KT, 128, 512), np.float32)
        for i, m in enumerate(
            (W_rel[s, :D], W_rel[s, D:], W_rel[s2, :D], W_rel[s2, D:])
        ):
            blk[:, :, i * 128 : (i + 1) * 128] = (
                np.ascontiguousarray(m.T).reshape(KT, 128, SL)
            )
        wrel = blk.transpose(1, 0, 2).reshape(128, KT * 512)

        sm = np.zeros((128, 31), np.float32)
        sm[:, 0:8] = b_enc.reshape(KT, 128).T
        sm[:, 8] = b_rel[s]
        sm[:, 9] = b_rel[s2]
        sm[:, 10:30] = gaussian[:, s].T
        sm[:, 30] = b_enc[s]
        in_maps.append(
            {
                "xsb": np.ascontiguousarray(
                    xs.transpose(1, 0, 2).reshape(128, -1)
                ).astype(nd),
                "wcm": np.ascontiguousarray(wcm).astype(nd),
                "wrel": np.ascontiguousarray(wrel).astype(nd),
                "sml": sm,
            }
        )
    return in_maps


def run(inputs: dict, trace: bool = False, use_ag: bool = USE_ALLGATHER):
    x = np.asarray(inputs["x_support"], np.float32)
    W_enc = np.asarray(inputs["W_enc"], np.float32)
    b_enc = np.asarray(inputs["b_enc"], np.float32)
    W_rel = np.asarray(inputs["W_rel"], np.float32)
    b_rel = np.asarray(inputs["b_rel"], np.float32)
    gaussian = np.asarray(inputs["gaussian_vectors"], np.float32)

    nc = _get_nc(use_ag)
    in_maps = _make_in_maps(x, W_enc, b_enc, W_rel, b_rel, gaussian, use_ag)
    res = run_bass_kernel_spmd(nc, in_maps, list(range(NC)), trace=trace)

    out = np.empty((N_WAY, D), np.float32)
    for c in range(NC):
        out[:, c * SL : (c + 1) * SL] = res.results[c]["out"].T
    return out, res


def kernel(**inputs) -> np.ndarray:
    out, _ = run(inputs)
    return out
